# revision 14
# baseline (speedup 1.0000x reference)
"""
Trainium2 Bass kernel for nn_DenseFeatureNumericEmbedding.

Computes, per feature f (F=128 independent tiny MLPs):
    h[b,f,:]   = relu(x[b,f] * w1[f,:] + b1[f,:])            # [B, F, H]
    out[b,f,:] = h[b,f,:] @ w2[f,:,:] + b2[f,:]              # [B, F, E]
    returns out.reshape(B, F*E)                              # [16384, 4096] fp32

Sharding: data-parallel over batch across 8 NeuronCores (2048 rows/core),
params replicated. No collectives; host concatenates the 8 output shards.

v5 dataflow:
 - NO on-device transpose: kernel writes outT [F*E, BL] bf16, host
   transposes + casts to fp32.
 - Quad-outer loop, all 4 batch chunks per quad: L1 stationaries are
   reused, PE matmul stream stays dense (HAM clock-gate friendly).
 - L1: per pair/chunk, 2 bf16 K=2 matmuls (bias folded via ones row),
   row-groups 32j -> pre [128,1024] fp32 PSUM; row-tiled pairs pack.
 - RELU ScalarE/VectorE split PSUM -> SBUF bf16.
 - L2: per chunk, 4 bf16 matmuls col-tiled (M=32) -> pout [128,512];
   software-pipelined: L2 of chunk c is emitted between later L1 chunks
   so the PE always has ready work while relu drains PSUM.
 - COPY +b2 fused (Identity+bias / tensor_scalar_add) -> outq [128,2048]
   staging; ONE output DMA per quad (4 KiB rows) on the Sync queue.
 - Input x DMAs ride the otherwise-idle GpSimd queue (descriptor
   generation for 128-row DMAs costs ~600ns each on the issuing queue).

PSUM: pre pool 3 bufs x 2 banks + pout 2 bufs x 1 bank = 8/8 banks.
"""

import sys

sys.path.insert(0, "/opt/trn_rl_repo")

import numpy as np
import ml_dtypes

import concourse.bass as bass
import concourse.tile as tile
from concourse import bacc, mybir
from concourse.bass_utils import run_bass_kernel_spmd

BF16 = ml_dtypes.bfloat16

B = 16384
F = 128
H = 128
E = 32
NCORES = 8
BL = B // NCORES          # 2048 rows per core
CHUNK = 512               # batch columns per inner tile (1 PSUM bank fp32)
NCHUNK = BL // CHUNK      # 4
NQUAD = F // 4            # 32 quads of 4 features

CONFIG = {
    "RELU_PAT": "ADADADADADADADADADADADADA",   # 13 A, 12 D per 25
    "COPY_PAT": "AD",
    "VARIANT_ID": 60,                          # busts the NEFF cache
}

_COMPILED = None


def _build_bass():
    nc = bacc.Bacc("TRN2", target_bir_lowering=False, debug=False,
                   num_devices=NCORES)
    dt = mybir.dt

    xt2 = nc.dram_tensor("xt2", [2 * F, BL], dt.bfloat16, kind="ExternalInput").ap()
    w1b1q = nc.dram_tensor("w1b1q", [128, F * H], dt.bfloat16, kind="ExternalInput").ap()
    w2s = nc.dram_tensor("w2s", [H, F * E], dt.bfloat16, kind="ExternalInput").ap()
    b2qs = nc.dram_tensor("b2qs", [128, NQUAD], dt.float32, kind="ExternalInput").ap()
    out = nc.dram_tensor("out", [F * E, BL], dt.bfloat16, kind="ExternalOutput").ap()

    # DRAM view of xt2: rows 8q + 2j + r (q quad, j feature-in-quad, r 0=x/1=ones)
    xt2_r = xt2.rearrange("(q g) n -> g q n", g=8)       # [8, NQUAD, BL]

    for _ in range(CONFIG["VARIANT_ID"]):
        nc.sync.nop()

    relu_pat = CONFIG["RELU_PAT"]
    copy_pat = CONFIG["COPY_PAT"]

    with tile.TileContext(nc) as tc:
        with (
            tc.tile_pool(name="params", bufs=1) as params,
            tc.tile_pool(name="xq", bufs=3) as xq_pool,
            tc.tile_pool(name="h", bufs=10) as h_pool,
            tc.tile_pool(name="outq", bufs=3) as outq_pool,
            tc.tile_pool(name="pre", bufs=3, space="PSUM") as pre_pool,
            tc.tile_pool(name="pout", bufs=2, space="PSUM") as pout_pool,
        ):
            # per-piece parameter tiles: Tile dependencies are tile-granular,
            # so quad q's matmuls wait only on the piece holding its slice
            b2_sb = params.tile([128, NQUAD], dt.float32, tag="b2qs")
            nc.sync.dma_start(out=b2_sb[:], in_=b2qs[:])
            NSPLIT = 8
            QPS = NQUAD // NSPLIT        # quads per piece
            w1b1_pc = []
            w2_pc = []
            for s in range(NSPLIT):
                t1 = params.tile([128, QPS * H], dt.bfloat16, tag=f"w1b1_{s}")
                nc.sync.dma_start(
                    out=t1[:], in_=w1b1q[:, bass.ts(s, QPS * H)])
                w1b1_pc.append(t1)
                t2 = params.tile([H, QPS * 4 * E], dt.bfloat16, tag=f"w2_{s}")
                nc.sync.dma_start(
                    out=t2[:], in_=w2s[:, bass.ts(s, QPS * 4 * E)])
                w2_pc.append(t2)

            relu_idx = 0
            copy_idx = 0

            def make_quad(q):
                # xqt rows 32j+r = [x; ones] of feature 4q+j over full BL
                xqt = xq_pool.tile([128, BL], dt.bfloat16, tag="xq")
                for j in range(4):
                    nc.gpsimd.dma_start(
                        out=xqt[32 * j:32 * j + 2, :],
                        in_=xt2_r[2 * j:2 * j + 2, q, :],
                    )
                hq = {}
                outq = outq_pool.tile([128, NCHUNK * CHUNK], dt.bfloat16,
                                      tag="outq")

                w1b1_t = w1b1_pc[q // QPS]
                w2_t = w2_pc[q // QPS]
                qr = q % QPS

                def do_l1(c):
                    nonlocal relu_idx
                    for p in range(2):      # pair p: features 4q+2p, 4q+2p+1
                        pre = pre_pool.tile([128, 2 * CHUNK], dt.float32,
                                            tag="pre")
                        for jj in range(2):
                            j = 2 * p + jj
                            nc.tensor.matmul(
                                pre[:, bass.ts(jj, CHUNK)],
                                lhsT=w1b1_t[32 * j:32 * j + 2,
                                            bass.ts(qr, H)],
                                rhs=xqt[32 * j:32 * j + 2,
                                        bass.ts(c, CHUNK)],
                                start=True, stop=True,
                                tile_position=(32 * j, 0),
                            )
                        hT = h_pool.tile([128, 2 * CHUNK], dt.bfloat16,
                                         tag="h")
                        if relu_pat[relu_idx % len(relu_pat)] == "A":
                            nc.scalar.activation(
                                hT[:], pre[:],
                                mybir.ActivationFunctionType.Relu)
                        else:
                            nc.vector.tensor_scalar_max(hT[:], pre[:], 0.0)
                        relu_idx += 1
                        hq[(p, c)] = hT

                def do_l2(c):
                    nonlocal copy_idx
                    pout = pout_pool.tile([128, CHUNK], dt.float32,
                                          tag="pout")
                    for j in range(4):
                        fr = 4 * qr + j
                        nc.tensor.matmul(
                            pout[32 * j:32 * j + 32, :],
                            lhsT=w2_t[:, bass.ts(fr, E)],
                            rhs=hq[(j // 2, c)][:, bass.ts(j % 2, CHUNK)],
                            start=True, stop=True,
                            tile_position=(0, 32 * j),
                        )
                    dst = outq[:, bass.ts(c, CHUNK)]
                    if copy_pat[copy_idx % len(copy_pat)] == "A":
                        nc.scalar.activation(
                            dst, pout[:],
                            mybir.ActivationFunctionType.Identity,
                            bias=b2_sb[:, q:q + 1],
                        )
                    else:
                        nc.vector.tensor_scalar_add(
                            dst, pout[:], b2_sb[:, q:q + 1])
                    copy_idx += 1

                def do_dma(half):
                    # half-quad output DMAs (256 KiB) on alternating queues
                    # so the tail transfer drains two DMA rings in parallel
                    eng = nc.sync if half == 0 else nc.gpsimd
                    eng.dma_start(
                        out=out[bass.ts(q, 128), bass.ts(half, 2 * CHUNK)],
                        in_=outq[:, bass.ts(half, 2 * CHUNK)])

                return do_l1, do_l2, do_dma

            # software pipeline across quads: the last L2 + output DMA of
            # quad q are deferred until after quad q+1's first L1 chunk, so
            # the PE always has L1 work ready behind the K=128 L2 matmuls
            # (whose row-group footprint blocks LDWEIGHTS pull-ahead).
            pending = None
            for q in range(NQUAD):
                do_l1, do_l2, do_dma = make_quad(q)
                do_l1(0)
                if pending is not None:
                    pl2, pdma = pending
                    pl2(3)
                    pdma(1)
                do_l1(1)
                do_l2(0)
                do_l1(2)
                do_l2(1)
                do_dma(0)
                do_l1(3)
                do_l2(2)
                pending = (do_l2, do_dma)
            pl2, pdma = pending
            pl2(3)
            pdma(1)

    nc.compile()
    return nc


def _prep_inputs(x, w1, b1, w2, b2):
    """Host-side packing of parameters + per-core x shards."""
    w1b1q = np.zeros((128, F * H), dtype=BF16)
    for f in range(F):
        q, j = divmod(f, 4)
        w1b1q[32 * j + 0, H * q:H * q + H] = w1[f].astype(BF16)
        w1b1q[32 * j + 1, H * q:H * q + H] = b1[f].astype(BF16)

    w2s = np.ascontiguousarray(
        w2.transpose(1, 0, 2).reshape(H, F * E)).astype(BF16)
    # b2qs[32j + e, q] = b2[4q + j, e]
    b2qs = np.ascontiguousarray(
        b2.reshape(NQUAD, 4, E).transpose(1, 2, 0).reshape(128, NQUAD)
    ).astype(np.float32)

    in_maps = []
    for core in range(NCORES):
        xs = x[core * BL:(core + 1) * BL]          # [BL, F]
        xt2 = np.empty((2 * F, BL), dtype=BF16)
        xt2[0::2] = xs.T.astype(BF16)
        xt2[1::2] = BF16(1.0)
        in_maps.append({
            "xt2": xt2, "w1b1q": w1b1q, "w2s": w2s, "b2qs": b2qs,
        })
    return in_maps


def _get_compiled():
    global _COMPILED
    if _COMPILED is None:
        _COMPILED = _build_bass()
    return _COMPILED


def reset_compiled():
    global _COMPILED
    _COMPILED = None


def kernel(x, w1, b1, w2, b2, _trace=False, _trace_kwargs=None):
    nc = _get_compiled()
    in_maps = _prep_inputs(
        np.asarray(x, dtype=np.float32), np.asarray(w1, dtype=np.float32),
        np.asarray(b1, dtype=np.float32), np.asarray(w2, dtype=np.float32),
        np.asarray(b2, dtype=np.float32))
    res = run_bass_kernel_spmd(
        nc, in_maps, core_ids=list(range(NCORES)),
        trace=_trace, **(_trace_kwargs or {}))
    # outT [F*E, BL] bf16 per core -> [BL, F*E] fp32, concatenated over cores
    shards = [
        np.asarray(res.results[i]["out"]).astype(np.float32).T
        for i in range(NCORES)
    ]
    full = np.ascontiguousarray(np.concatenate(shards, axis=0))
    if _trace:
        return full, res
    return full


if __name__ == "__main__":
    rng = np.random.default_rng(0)
    x = rng.standard_normal((B, F), dtype=np.float32)
    w1 = rng.standard_normal((F, H), dtype=np.float32)
    b1 = rng.standard_normal((F, H), dtype=np.float32)
    w2 = (rng.standard_normal((F, H, E), dtype=np.float32) / np.sqrt(H)).astype(np.float32)
    b2 = rng.standard_normal((F, E), dtype=np.float32) / np.sqrt(H)
    got = kernel(x=x, w1=w1, b1=b1, w2=w2, b2=b2)
    h = np.maximum(x[:, :, None] * w1[None] + b1[None], 0.0)
    want = (np.einsum("bfh,fhe->bfe", h, w2) + b2[None]).reshape(B, F * E)
    err = np.abs(got - want).max() / np.abs(want).max()
    print("self-test scale-relative max err:", err)


# revision 15
# speedup vs baseline: 1.1300x; 1.1300x over previous
"""
Trainium2 Bass kernel for nn_DenseFeatureNumericEmbedding.

Computes, per feature f (F=128 independent tiny MLPs):
    h[b,f,:]   = relu(x[b,f] * w1[f,:] + b1[f,:])            # [B, F, H]
    out[b,f,:] = h[b,f,:] @ w2[f,:,:] + b2[f,:]              # [B, F, E]
    returns out.reshape(B, F*E)                              # [16384, 4096] fp32

Sharding: data-parallel over batch across 8 NeuronCores (2048 rows/core),
params replicated. No collectives; host concatenates the 8 output shards.

v5 dataflow:
 - NO on-device transpose: kernel writes outT [F*E, BL] bf16, host
   transposes + casts to fp32.
 - Quad-outer loop, all 4 batch chunks per quad: L1 stationaries are
   reused, PE matmul stream stays dense (HAM clock-gate friendly).
 - L1: per pair/chunk, 2 bf16 K=2 matmuls (bias folded via ones row),
   row-groups 32j -> pre [128,1024] fp32 PSUM; row-tiled pairs pack.
 - RELU ScalarE/VectorE split PSUM -> SBUF bf16.
 - L2: per chunk, 4 bf16 matmuls col-tiled (M=32) -> pout [128,512];
   software-pipelined: L2 of chunk c is emitted between later L1 chunks
   so the PE always has ready work while relu drains PSUM.
 - COPY +b2 fused (Identity+bias / tensor_scalar_add) -> outq [128,2048]
   staging; ONE output DMA per quad (4 KiB rows) on the Sync queue.
 - Input x DMAs ride the otherwise-idle GpSimd queue (descriptor
   generation for 128-row DMAs costs ~600ns each on the issuing queue).

PSUM: pre pool 3 bufs x 2 banks + pout 2 bufs x 1 bank = 8/8 banks.
"""

import sys

sys.path.insert(0, "/opt/trn_rl_repo")

import numpy as np
import ml_dtypes

import concourse.bass as bass
import concourse.tile as tile
from concourse import bacc, mybir
from concourse.bass_utils import run_bass_kernel_spmd

BF16 = ml_dtypes.bfloat16

B = 16384
F = 128
H = 128
E = 32
NCORES = 8
BL = B // NCORES          # 2048 rows per core
CHUNK = 512               # batch columns per inner tile (1 PSUM bank fp32)
NCHUNK = BL // CHUNK      # 4
NQUAD = F // 4            # 32 quads of 4 features

CONFIG = {
    "RELU_PAT": "ADADADADADADADADADADADADADADADAA",  # 17 A, 15 D per 32
    "COPY_PAT": "AD",
    "VARIANT_ID": 70,                          # busts the NEFF cache
}

_COMPILED = None


def _build_bass():
    nc = bacc.Bacc("TRN2", target_bir_lowering=False, debug=False,
                   num_devices=NCORES)
    dt = mybir.dt

    xt2 = nc.dram_tensor("xt2", [2 * F, BL], dt.bfloat16, kind="ExternalInput").ap()
    w1b1q = nc.dram_tensor("w1b1q", [128, F * H], dt.bfloat16, kind="ExternalInput").ap()
    w2s = nc.dram_tensor("w2s", [H, F * E], dt.bfloat16, kind="ExternalInput").ap()
    b2qs = nc.dram_tensor("b2qs", [128, NQUAD], dt.float32, kind="ExternalInput").ap()
    out = nc.dram_tensor("out", [F * E, BL], dt.bfloat16, kind="ExternalOutput").ap()

    # DRAM view of xt2: rows 8q + 2j + r (q quad, j feature-in-quad, r 0=x/1=ones)
    xt2_r = xt2.rearrange("(q g) n -> g q n", g=8)       # [8, NQUAD, BL]

    for _ in range(CONFIG["VARIANT_ID"]):
        nc.sync.nop()

    relu_pat = CONFIG["RELU_PAT"]
    copy_pat = CONFIG["COPY_PAT"]

    with tile.TileContext(nc) as tc:
        with (
            tc.tile_pool(name="params", bufs=1) as params,
            tc.tile_pool(name="xq", bufs=3) as xq_pool,
            tc.tile_pool(name="h", bufs=10) as h_pool,
            tc.tile_pool(name="outq", bufs=3) as outq_pool,
            tc.tile_pool(name="pre", bufs=3, space="PSUM") as pre_pool,
            tc.tile_pool(name="pout", bufs=2, space="PSUM") as pout_pool,
        ):
            # per-piece parameter tiles: Tile dependencies are tile-granular,
            # so quad q's matmuls wait only on the piece holding its slice
            b2_sb = params.tile([128, NQUAD], dt.float32, tag="b2qs")
            nc.sync.dma_start(out=b2_sb[:], in_=b2qs[:])
            NSPLIT = 8
            QPS = NQUAD // NSPLIT        # quads per piece
            w1b1_pc = []
            w2_pc = []
            for s in range(NSPLIT):
                t1 = params.tile([128, QPS * H], dt.bfloat16, tag=f"w1b1_{s}")
                nc.sync.dma_start(
                    out=t1[:], in_=w1b1q[:, bass.ts(s, QPS * H)])
                w1b1_pc.append(t1)
                t2 = params.tile([H, QPS * 4 * E], dt.bfloat16, tag=f"w2_{s}")
                nc.sync.dma_start(
                    out=t2[:], in_=w2s[:, bass.ts(s, QPS * 4 * E)])
                w2_pc.append(t2)

            relu_idx = 0
            copy_idx = 0

            def make_quad(q):
                # xqt rows 32j+r = [x; ones] of feature 4q+j over full BL
                xqt = xq_pool.tile([128, BL], dt.bfloat16, tag="xq")
                for j in range(4):
                    nc.gpsimd.dma_start(
                        out=xqt[32 * j:32 * j + 2, :],
                        in_=xt2_r[2 * j:2 * j + 2, q, :],
                    )
                hq = {}
                outq = outq_pool.tile([128, NCHUNK * CHUNK], dt.bfloat16,
                                      tag="outq")

                w1b1_t = w1b1_pc[q // QPS]
                w2_t = w2_pc[q // QPS]
                qr = q % QPS

                def do_l1(c):
                    nonlocal relu_idx
                    for p in range(2):      # pair p: features 4q+2p, 4q+2p+1
                        pre = pre_pool.tile([128, 2 * CHUNK], dt.float32,
                                            tag="pre")
                        for jj in range(2):
                            j = 2 * p + jj
                            nc.tensor.matmul(
                                pre[:, bass.ts(jj, CHUNK)],
                                lhsT=w1b1_t[32 * j:32 * j + 2,
                                            bass.ts(qr, H)],
                                rhs=xqt[32 * j:32 * j + 2,
                                        bass.ts(c, CHUNK)],
                                start=True, stop=True,
                                tile_position=(32 * j, 0),
                            )
                        hT = h_pool.tile([128, 2 * CHUNK], dt.bfloat16,
                                         tag="h")
                        if relu_pat[relu_idx % len(relu_pat)] == "A":
                            nc.scalar.activation(
                                hT[:], pre[:],
                                mybir.ActivationFunctionType.Relu)
                        else:
                            nc.vector.tensor_scalar_max(hT[:], pre[:], 0.0)
                        relu_idx += 1
                        hq[(p, c)] = hT

                def do_l2(c):
                    nonlocal copy_idx
                    pout = pout_pool.tile([128, CHUNK], dt.float32,
                                          tag="pout")
                    for j in (0, 2, 1, 3):
                        fr = 4 * qr + j
                        nc.tensor.matmul(
                            pout[32 * j:32 * j + 32, :],
                            lhsT=w2_t[:, bass.ts(fr, E)],
                            rhs=hq[(j // 2, c)][:, bass.ts(j % 2, CHUNK)],
                            start=True, stop=True,
                            tile_position=(0, 32 * j),
                        )
                    dst = outq[:, bass.ts(c, CHUNK)]
                    if copy_pat[copy_idx % len(copy_pat)] == "A":
                        nc.scalar.activation(
                            dst, pout[:],
                            mybir.ActivationFunctionType.Identity,
                            bias=b2_sb[:, q:q + 1],
                        )
                    else:
                        nc.vector.tensor_scalar_add(
                            dst, pout[:], b2_sb[:, q:q + 1])
                    copy_idx += 1

                def do_dma(half):
                    # half-quad output DMAs (256 KiB) on alternating queues
                    # so the tail transfer drains two DMA rings in parallel
                    nc.sync.dma_start(
                        out=out[bass.ts(q, 128), bass.ts(half, 2 * CHUNK)],
                        in_=outq[:, bass.ts(half, 2 * CHUNK)])

                return do_l1, do_l2, do_dma

            # software pipeline across quads: the last L2 + output DMA of
            # quad q are deferred until after quad q+1's first L1 chunk, so
            # the PE always has L1 work ready behind the K=128 L2 matmuls
            # (whose row-group footprint blocks LDWEIGHTS pull-ahead).
            pending = None
            for q in range(NQUAD):
                do_l1, do_l2, do_dma = make_quad(q)
                do_l1(0)
                if pending is not None:
                    pl2, pdma = pending
                    pl2(3)
                    pdma(1)
                do_l1(1)
                do_l2(0)
                do_l1(2)
                do_l2(1)
                do_dma(0)
                do_l1(3)
                do_l2(2)
                pending = (do_l2, do_dma)
            pl2, pdma = pending
            pl2(3)
            pdma(1)

    nc.compile()
    return nc


def _prep_inputs(x, w1, b1, w2, b2):
    """Host-side packing of parameters + per-core x shards."""
    w1b1q = np.zeros((128, F * H), dtype=BF16)
    for f in range(F):
        q, j = divmod(f, 4)
        w1b1q[32 * j + 0, H * q:H * q + H] = w1[f].astype(BF16)
        w1b1q[32 * j + 1, H * q:H * q + H] = b1[f].astype(BF16)

    w2s = np.ascontiguousarray(
        w2.transpose(1, 0, 2).reshape(H, F * E)).astype(BF16)
    # b2qs[32j + e, q] = b2[4q + j, e]
    b2qs = np.ascontiguousarray(
        b2.reshape(NQUAD, 4, E).transpose(1, 2, 0).reshape(128, NQUAD)
    ).astype(np.float32)

    in_maps = []
    for core in range(NCORES):
        xs = x[core * BL:(core + 1) * BL]          # [BL, F]
        xt2 = np.empty((2 * F, BL), dtype=BF16)
        xt2[0::2] = xs.T.astype(BF16)
        xt2[1::2] = BF16(1.0)
        in_maps.append({
            "xt2": xt2, "w1b1q": w1b1q, "w2s": w2s, "b2qs": b2qs,
        })
    return in_maps


def _get_compiled():
    global _COMPILED
    if _COMPILED is None:
        _COMPILED = _build_bass()
    return _COMPILED


def reset_compiled():
    global _COMPILED
    _COMPILED = None


def kernel(x, w1, b1, w2, b2, _trace=False, _trace_kwargs=None):
    nc = _get_compiled()
    in_maps = _prep_inputs(
        np.asarray(x, dtype=np.float32), np.asarray(w1, dtype=np.float32),
        np.asarray(b1, dtype=np.float32), np.asarray(w2, dtype=np.float32),
        np.asarray(b2, dtype=np.float32))
    res = run_bass_kernel_spmd(
        nc, in_maps, core_ids=list(range(NCORES)),
        trace=_trace, **(_trace_kwargs or {}))
    # outT [F*E, BL] bf16 per core -> [BL, F*E] fp32, concatenated over cores
    shards = [
        np.asarray(res.results[i]["out"]).astype(np.float32).T
        for i in range(NCORES)
    ]
    full = np.ascontiguousarray(np.concatenate(shards, axis=0))
    if _trace:
        return full, res
    return full


if __name__ == "__main__":
    rng = np.random.default_rng(0)
    x = rng.standard_normal((B, F), dtype=np.float32)
    w1 = rng.standard_normal((F, H), dtype=np.float32)
    b1 = rng.standard_normal((F, H), dtype=np.float32)
    w2 = (rng.standard_normal((F, H, E), dtype=np.float32) / np.sqrt(H)).astype(np.float32)
    b2 = rng.standard_normal((F, E), dtype=np.float32) / np.sqrt(H)
    got = kernel(x=x, w1=w1, b1=b1, w2=w2, b2=b2)
    h = np.maximum(x[:, :, None] * w1[None] + b1[None], 0.0)
    want = (np.einsum("bfh,fhe->bfe", h, w2) + b2[None]).reshape(B, F * E)
    err = np.abs(got - want).max() / np.abs(want).max()
    print("self-test scale-relative max err:", err)


# revision 16
# speedup vs baseline: 1.1408x; 1.0096x over previous
"""
Trainium2 Bass kernel for nn_DenseFeatureNumericEmbedding.

Computes, per feature f (F=128 independent tiny MLPs):
    h[b,f,:]   = relu(x[b,f] * w1[f,:] + b1[f,:])            # [B, F, H]
    out[b,f,:] = h[b,f,:] @ w2[f,:,:] + b2[f,:]              # [B, F, E]
    returns out.reshape(B, F*E)                              # [16384, 4096] fp32

Sharding: data-parallel over batch across 8 NeuronCores (2048 rows/core),
params replicated. No collectives; host concatenates the 8 output shards.

v5 dataflow:
 - NO on-device transpose: kernel writes outT [F*E, BL] bf16, host
   transposes + casts to fp32.
 - Quad-outer loop, all 4 batch chunks per quad: L1 stationaries are
   reused, PE matmul stream stays dense (HAM clock-gate friendly).
 - L1: per pair/chunk, 2 bf16 K=2 matmuls (bias folded via ones row),
   row-groups 32j -> pre [128,1024] fp32 PSUM; row-tiled pairs pack.
 - RELU ScalarE/VectorE split PSUM -> SBUF bf16.
 - L2: per chunk, 4 bf16 matmuls col-tiled (M=32) -> pout [128,512];
   software-pipelined: L2 of chunk c is emitted between later L1 chunks
   so the PE always has ready work while relu drains PSUM.
 - COPY +b2 fused (Identity+bias / tensor_scalar_add) -> outq [128,2048]
   staging; ONE output DMA per quad (4 KiB rows) on the Sync queue.
 - Input x DMAs ride the otherwise-idle GpSimd queue (descriptor
   generation for 128-row DMAs costs ~600ns each on the issuing queue).

PSUM: pre pool 3 bufs x 2 banks + pout 2 bufs x 1 bank = 8/8 banks.
"""

import sys

sys.path.insert(0, "/opt/trn_rl_repo")

import numpy as np
import ml_dtypes

import concourse.bass as bass
import concourse.tile as tile
from concourse import bacc, mybir
from concourse.bass_utils import run_bass_kernel_spmd

BF16 = ml_dtypes.bfloat16

B = 16384
F = 128
H = 128
E = 32
NCORES = 8
BL = B // NCORES          # 2048 rows per core
CHUNK = 512               # batch columns per inner tile (1 PSUM bank fp32)
NCHUNK = BL // CHUNK      # 4
NQUAD = F // 4            # 32 quads of 4 features

CONFIG = {
    "RELU_PAT": "ADADADADADADADADADADADADADADADAA",  # 17 A, 15 D per 32
    "COPY_PAT": "AD",
    "VARIANT_ID": 80,                          # busts the NEFF cache
}

_COMPILED = None


def _build_bass():
    nc = bacc.Bacc("TRN2", target_bir_lowering=False, debug=False,
                   num_devices=NCORES)
    dt = mybir.dt

    xt2 = nc.dram_tensor("xt2", [2 * F, BL], dt.bfloat16, kind="ExternalInput").ap()
    w1b1q = nc.dram_tensor("w1b1q", [128, F * H], dt.bfloat16, kind="ExternalInput").ap()
    w2s = nc.dram_tensor("w2s", [H, F * E], dt.bfloat16, kind="ExternalInput").ap()
    b2qs = nc.dram_tensor("b2qs", [128, NQUAD], dt.float32, kind="ExternalInput").ap()
    out = nc.dram_tensor("out", [F * E, BL], dt.bfloat16, kind="ExternalOutput").ap()

    # DRAM view of xt2: rows 8q + 2j + r (q quad, j feature-in-quad, r 0=x/1=ones)
    xt2_r = xt2.rearrange("(q g) n -> g q n", g=8)       # [8, NQUAD, BL]

    for _ in range(CONFIG["VARIANT_ID"]):
        nc.sync.nop()

    relu_pat = CONFIG["RELU_PAT"]
    copy_pat = CONFIG["COPY_PAT"]

    with tile.TileContext(nc) as tc:
        with (
            tc.tile_pool(name="params", bufs=1) as params,
            tc.tile_pool(name="xq", bufs=3) as xq_pool,
            tc.tile_pool(name="h", bufs=10) as h_pool,
            tc.tile_pool(name="outq", bufs=3) as outq_pool,
            tc.tile_pool(name="pre", bufs=3, space="PSUM") as pre_pool,
            tc.tile_pool(name="pout", bufs=2, space="PSUM") as pout_pool,
        ):
            # per-piece parameter tiles: Tile dependencies are tile-granular,
            # so quad q's matmuls wait only on the piece holding its slice
            b2_sb = params.tile([128, NQUAD], dt.float32, tag="b2qs")
            nc.sync.dma_start(out=b2_sb[:], in_=b2qs[:])
            NSPLIT = 16
            QPS = NQUAD // NSPLIT        # quads per piece
            w1b1_pc = []
            w2_pc = []
            for s in range(NSPLIT):
                t1 = params.tile([128, QPS * H], dt.bfloat16, tag=f"w1b1_{s}")
                nc.sync.dma_start(
                    out=t1[:], in_=w1b1q[:, bass.ts(s, QPS * H)])
                w1b1_pc.append(t1)
                t2 = params.tile([H, QPS * 4 * E], dt.bfloat16, tag=f"w2_{s}")
                nc.sync.dma_start(
                    out=t2[:], in_=w2s[:, bass.ts(s, QPS * 4 * E)])
                w2_pc.append(t2)

            relu_idx = 0
            copy_idx = 0

            def make_quad(q):
                # xqt rows 32j+r = [x; ones] of feature 4q+j over full BL
                xqt = xq_pool.tile([128, BL], dt.bfloat16, tag="xq")
                for j in range(4):
                    nc.gpsimd.dma_start(
                        out=xqt[32 * j:32 * j + 2, :],
                        in_=xt2_r[2 * j:2 * j + 2, q, :],
                    )
                hq = {}
                outq = outq_pool.tile([128, NCHUNK * CHUNK], dt.bfloat16,
                                      tag="outq")

                w1b1_t = w1b1_pc[q // QPS]
                w2_t = w2_pc[q // QPS]
                qr = q % QPS

                def do_l1(c):
                    nonlocal relu_idx
                    for p in range(2):      # pair p: features 4q+2p, 4q+2p+1
                        pre = pre_pool.tile([128, 2 * CHUNK], dt.float32,
                                            tag="pre")
                        for jj in range(2):
                            j = 2 * p + jj
                            nc.tensor.matmul(
                                pre[:, bass.ts(jj, CHUNK)],
                                lhsT=w1b1_t[32 * j:32 * j + 2,
                                            bass.ts(qr, H)],
                                rhs=xqt[32 * j:32 * j + 2,
                                        bass.ts(c, CHUNK)],
                                start=True, stop=True,
                                tile_position=(32 * j, 0),
                            )
                        hT = h_pool.tile([128, 2 * CHUNK], dt.bfloat16,
                                         tag="h")
                        if relu_pat[relu_idx % len(relu_pat)] == "A":
                            nc.scalar.activation(
                                hT[:], pre[:],
                                mybir.ActivationFunctionType.Relu)
                        else:
                            nc.vector.tensor_scalar_max(hT[:], pre[:], 0.0)
                        relu_idx += 1
                        hq[(p, c)] = hT

                def do_l2(c):
                    nonlocal copy_idx
                    pout = pout_pool.tile([128, CHUNK], dt.float32,
                                          tag="pout")
                    for j in (0, 2, 1, 3):
                        fr = 4 * qr + j
                        nc.tensor.matmul(
                            pout[32 * j:32 * j + 32, :],
                            lhsT=w2_t[:, bass.ts(fr, E)],
                            rhs=hq[(j // 2, c)][:, bass.ts(j % 2, CHUNK)],
                            start=True, stop=True,
                            tile_position=(0, 32 * j),
                        )
                    dst = outq[:, bass.ts(c, CHUNK)]
                    if copy_pat[copy_idx % len(copy_pat)] == "A":
                        nc.scalar.activation(
                            dst, pout[:],
                            mybir.ActivationFunctionType.Identity,
                            bias=b2_sb[:, q:q + 1],
                        )
                    else:
                        nc.vector.tensor_scalar_add(
                            dst, pout[:], b2_sb[:, q:q + 1])
                    copy_idx += 1

                def do_dma(half):
                    # per-chunk output DMAs (128 KiB) so the tail transfer
                    # drains quickly and DMA load spreads across rings
                    for cc in range(2):
                        c = 2 * half + cc
                        nc.sync.dma_start(
                            out=out[bass.ts(q, 128), bass.ts(c, CHUNK)],
                            in_=outq[:, bass.ts(c, CHUNK)])

                return do_l1, do_l2, do_dma

            # software pipeline across quads: the last L2 + output DMA of
            # quad q are deferred until after quad q+1's first L1 chunk, so
            # the PE always has L1 work ready behind the K=128 L2 matmuls
            # (whose row-group footprint blocks LDWEIGHTS pull-ahead).
            pending = None
            for q in range(NQUAD):
                do_l1, do_l2, do_dma = make_quad(q)
                do_l1(0)
                if pending is not None:
                    pl2, pdma = pending
                    pl2(3)
                    pdma(1)
                do_l1(1)
                do_l2(0)
                do_l1(2)
                do_l2(1)
                do_dma(0)
                do_l1(3)
                do_l2(2)
                pending = (do_l2, do_dma)
            pl2, pdma = pending
            pl2(3)
            pdma(1)

    nc.compile()
    return nc


def _prep_inputs(x, w1, b1, w2, b2):
    """Host-side packing of parameters + per-core x shards."""
    w1b1q = np.zeros((128, F * H), dtype=BF16)
    for f in range(F):
        q, j = divmod(f, 4)
        w1b1q[32 * j + 0, H * q:H * q + H] = w1[f].astype(BF16)
        w1b1q[32 * j + 1, H * q:H * q + H] = b1[f].astype(BF16)

    w2s = np.ascontiguousarray(
        w2.transpose(1, 0, 2).reshape(H, F * E)).astype(BF16)
    # b2qs[32j + e, q] = b2[4q + j, e]
    b2qs = np.ascontiguousarray(
        b2.reshape(NQUAD, 4, E).transpose(1, 2, 0).reshape(128, NQUAD)
    ).astype(np.float32)

    in_maps = []
    for core in range(NCORES):
        xs = x[core * BL:(core + 1) * BL]          # [BL, F]
        xt2 = np.empty((2 * F, BL), dtype=BF16)
        xt2[0::2] = xs.T.astype(BF16)
        xt2[1::2] = BF16(1.0)
        in_maps.append({
            "xt2": xt2, "w1b1q": w1b1q, "w2s": w2s, "b2qs": b2qs,
        })
    return in_maps


def _get_compiled():
    global _COMPILED
    if _COMPILED is None:
        _COMPILED = _build_bass()
    return _COMPILED


def reset_compiled():
    global _COMPILED
    _COMPILED = None


def kernel(x, w1, b1, w2, b2, _trace=False, _trace_kwargs=None):
    nc = _get_compiled()
    in_maps = _prep_inputs(
        np.asarray(x, dtype=np.float32), np.asarray(w1, dtype=np.float32),
        np.asarray(b1, dtype=np.float32), np.asarray(w2, dtype=np.float32),
        np.asarray(b2, dtype=np.float32))
    res = run_bass_kernel_spmd(
        nc, in_maps, core_ids=list(range(NCORES)),
        trace=_trace, **(_trace_kwargs or {}))
    # outT [F*E, BL] bf16 per core -> [BL, F*E] fp32, concatenated over cores
    shards = [
        np.asarray(res.results[i]["out"]).astype(np.float32).T
        for i in range(NCORES)
    ]
    full = np.ascontiguousarray(np.concatenate(shards, axis=0))
    if _trace:
        return full, res
    return full


if __name__ == "__main__":
    rng = np.random.default_rng(0)
    x = rng.standard_normal((B, F), dtype=np.float32)
    w1 = rng.standard_normal((F, H), dtype=np.float32)
    b1 = rng.standard_normal((F, H), dtype=np.float32)
    w2 = (rng.standard_normal((F, H, E), dtype=np.float32) / np.sqrt(H)).astype(np.float32)
    b2 = rng.standard_normal((F, E), dtype=np.float32) / np.sqrt(H)
    got = kernel(x=x, w1=w1, b1=b1, w2=w2, b2=b2)
    h = np.maximum(x[:, :, None] * w1[None] + b1[None], 0.0)
    want = (np.einsum("bfh,fhe->bfe", h, w2) + b2[None]).reshape(B, F * E)
    err = np.abs(got - want).max() / np.abs(want).max()
    print("self-test scale-relative max err:", err)
